# revision 4
# baseline (speedup 1.0000x reference)
"""Sparse-attention Trainium2 kernel (nn_Attention_81398220193933).

Strategy (tensor-parallel over heads, 2 heads per NeuronCore):
  - Host pre-lays-out per-core tensors:
      qT  [B, 128, S]  bf16 : rows 0:64 = headA Q^T / sqrt(dh), rows 64:128 = headB
      kT  [B, 128, S]  bf16 : same for K^T
      vE  [B, 128, 8, 130] bf16 : per k-tile t, partition p = key position t*128+p,
           cols [0:64]=V_A*emb, [64]=emb, [65:129]=V_B*emb, [129]=emb
           where emb[b,k] = exp(bias[k]) * (k < seq_len[b]) (all-valid if seq_len==0).
    Folding the additive key bias + mask multiplicatively into V makes the
    softmax mask/bias free on-device and lets fully-masked k-tiles be skipped.
  - Device, per batch b and key-tile t (Kb = ceil(seq_len/128) tiles):
      scores^T [k=128, q=1024] = K_tile^T.T @ Q^T  (two heads packed into the
          PE array as 64-row tile_position groups -> run concurrently)
      W^T = exp(scores^T) on ScalarE (PSUM -> SBUF, bf16)
      out[q,65] += W^T_chunk.T @ V_tile  accumulated over t in PSUM; column 64
          accumulates the softmax denominator (via the emb column of vE).
    Epilogue per batch: denominators -> DVE reciprocal -> per-partition
    tensor_scalar multiply -> out [q, d] f32 -> DMA to HBM.
  - Softmax max-subtraction is unnecessary: logits are O(+-6) and masked keys
    contribute exactly zero through emb; a fully-masked row degenerates to
    softmax over all keys exactly like the jax reference (the -1e12 shift
    cancels there).
"""

import numpy as np
import ml_dtypes

import concourse.bass as bass
import concourse.mybir as mybir
import concourse.tile as tile
from concourse import bacc
from concourse.bass_utils import run_bass_kernel_spmd

B = 8
S = 1024
UNITS = 1024
H = 16
DH = 64
N_CORES = 8
KT = S // 128  # max key tiles per batch

BF16 = mybir.dt.bfloat16
F32 = mybir.dt.float32


def _build_nc(kbs):
    """Build the SPMD Bass program. kbs: per-batch number of 128-key tiles."""
    nc = bacc.Bacc("TRN2", target_bir_lowering=False, debug=False,
                   num_devices=N_CORES)
    qT = nc.dram_tensor("qt", [B, 128, S], BF16, kind="ExternalInput").ap()
    kT = nc.dram_tensor("kt", [B, 128, S], BF16, kind="ExternalInput").ap()
    vE = nc.dram_tensor("vt", [B, 128, KT, 130], BF16, kind="ExternalInput").ap()
    o = nc.dram_tensor("o", [B, S, 128], F32, kind="ExternalOutput").ap()

    with tile.TileContext(nc) as tc:
        with (
            tc.tile_pool(name="qk", bufs=2) as qk_pool,
            tc.tile_pool(name="v", bufs=2) as v_pool,
            tc.tile_pool(name="w", bufs=32) as w_pool,
            tc.tile_pool(name="ot", bufs=2) as o_pool,
            tc.tile_pool(name="rc", bufs=4) as r_pool,
            tc.tile_pool(name="sc", bufs=2, space="PSUM") as sc_pool,
            tc.tile_pool(name="acc", bufs=4, space="PSUM") as acc_pool,
        ):
            for b in range(B):
                kb = kbs[b]
                qt = qk_pool.tile([128, S], BF16, tag="qt")
                nc.sync.dma_start(out=qt[:], in_=qT[b])
                kt = qk_pool.tile([128, S], BF16, tag="kt")
                nc.sync.dma_start(out=kt[:], in_=kT[b])
                vt = v_pool.tile([128, kb, 130], BF16, tag="vt")
                nc.sync.dma_start(out=vt[:], in_=vE[b, :, :kb, :])

                # PSUM accumulators: [headA q0-3, headA q4-7, headB q0-3, headB q4-7]
                out_ps = [acc_pool.tile([128, 4, 65], F32, tag="acc",
                                        name=f"acc{i}")
                          for i in range(4)]

                wts = [[], []]
                for t in range(kb):
                    for h in range(2):
                        base = 64 * h
                        sc = sc_pool.tile([128, S], F32, tag="sc")
                        for qc in range(2):
                            nc.tensor.matmul(
                                sc[:, qc * 512:(qc + 1) * 512],
                                lhsT=kt[base:base + 64, t * 128:(t + 1) * 128],
                                rhs=qt[base:base + 64, qc * 512:(qc + 1) * 512],
                                start=True, stop=True,
                            )
                        wt = w_pool.tile([128, S], BF16, tag="w",
                                         name=f"w{b}_{t}_{h}")
                        nc.scalar.activation(wt[:], sc[:],
                                             mybir.ActivationFunctionType.Exp)
                        wts[h].append(wt)
                # A-V matmuls: j-outer / t-inner so each PSUM accumulation
                # group closes before the next one opens in the same bank.
                for h in range(2):
                    for j in range(8):
                        grp = out_ps[2 * h + j // 4]
                        for t in range(kb):
                            nc.tensor.matmul(
                                grp[:, j % 4, :],
                                lhsT=wts[h][t][:, j * 128:(j + 1) * 128],
                                rhs=vt[:, t, h * 65:h * 65 + 65],
                                start=(t == 0), stop=(t == kb - 1),
                            )

                # epilogue: divide by denominator, assemble [q, (tile, dA|dB)]
                ot = o_pool.tile([128, 8, 128], F32, tag="ot")
                for h in range(2):
                    for g in range(2):
                        grp = out_ps[2 * h + g]
                        rc = r_pool.tile([128, 4, 1], F32, tag="rc")
                        nc.vector.reciprocal(rc[:], grp[:, :, 64:65])
                        for jj in range(4):
                            j = 4 * g + jj
                            nc.vector.tensor_scalar_mul(
                                ot[:, j, h * 64:(h + 1) * 64],
                                grp[:, jj, 0:64],
                                rc[:, jj, :],
                            )
                nc.sync.dma_start(
                    out=o[b].rearrange("(t p) c -> p t c", p=128),
                    in_=ot[:],
                )
    nc.compile()
    return nc


_NC_CACHE = {}


def _get_nc(kbs):
    key = tuple(kbs)
    if key not in _NC_CACHE:
        _NC_CACHE[key] = _build_nc(key)
    return _NC_CACHE[key]


def kernel(memory, query, b, seq_len):
    memory = np.asarray(memory)
    query = np.asarray(query)
    bias = np.asarray(b, dtype=np.float32)
    seq_len = np.asarray(seq_len).reshape(-1).astype(np.int64)

    sl = seq_len.copy()
    kbs = [int(min(KT, max(1, -(-int(s) // 128)))) if s > 0 else KT for s in sl]

    # emb[b, k] = exp(bias[k]) * valid; fully-masked batch -> plain softmax
    pos = np.arange(S)[None, :]
    valid = (pos < sl[:, None]) | (sl[:, None] == 0)
    emb = np.exp(bias)[None, :] * valid.astype(np.float32)  # [B, S]

    qh = (query.astype(np.float32) * (DH ** -0.5)).reshape(B, S, H, DH)
    kh = memory[:, :, :UNITS].astype(np.float32).reshape(B, S, H, DH)
    vh = memory[:, :, UNITS:].astype(np.float32).reshape(B, S, H, DH)
    vh = vh * emb[:, :, None, None]  # [B, S, H, DH] value rows pre-masked

    bf = ml_dtypes.bfloat16
    # [B, S, H, DH] -> [B, H, DH, S] transposed layouts
    qTfull = np.ascontiguousarray(qh.transpose(0, 2, 3, 1)).astype(bf)
    kTfull = np.ascontiguousarray(kh.transpose(0, 2, 3, 1)).astype(bf)
    # [B, S, H, DH] -> [B, (t p), H, DH] -> [B, 128, KT, H, DH]
    vtiles = np.ascontiguousarray(
        vh.reshape(B, KT, 128, H, DH).transpose(0, 2, 1, 3, 4)).astype(bf)
    embt = np.ascontiguousarray(
        emb.reshape(B, KT, 128).transpose(0, 2, 1)).astype(bf)  # [B, 128, KT]

    in_maps = []
    for c in range(N_CORES):
        hA, hB = 2 * c, 2 * c + 1
        qT = np.concatenate([qTfull[:, hA], qTfull[:, hB]], axis=1)  # [B,128,S]
        kT = np.concatenate([kTfull[:, hA], kTfull[:, hB]], axis=1)
        vE = np.empty((B, 128, KT, 130), dtype=bf)
        vE[..., 0:64] = vtiles[:, :, :, hA, :]
        vE[..., 64] = embt
        vE[..., 65:129] = vtiles[:, :, :, hB, :]
        vE[..., 129] = embt
        in_maps.append({
            "qt": np.ascontiguousarray(qT),
            "kt": np.ascontiguousarray(kT),
            "vt": np.ascontiguousarray(vE),
        })

    nc = _get_nc(kbs)
    res = run_bass_kernel_spmd(nc, in_maps, core_ids=list(range(N_CORES)))

    out = np.empty((B, S, UNITS), dtype=np.float32)
    for c in range(N_CORES):
        out[:, :, 128 * c:128 * (c + 1)] = res.results[c]["o"]
    return out
